# revision 12
# baseline (speedup 1.0000x reference)
"""DigitCaps kernel for 8 Trainium2 NeuronCores (fp16, (n,i)-layout).

Math (per batch b):
    U_hat[b,d,n,j] = sum_i W[d,n,j,i] * u[b,n,i]
    A_sum[b,d,m]   = s[b,d,:] . U_hat[b,d,m,:] / sqrt(dp),  s = sum_n U_hat
    C              = softmax_d(A_sum)
    S[b,d,j]       = sum_m (B_prior[d,m] + C[b,d,m]) * U_hat[b,d,m,j]
    out            = squash(S)

Sharding: data-parallel over batch, 2 batches per core, W/B_prior replicated.

Key layout trick: W tiles are host-arranged with partitions = (n16, i8) so
the per-(chunk,batch) vote products are single tensor_scalar ops (fp16 4x
mode on DVE; ACT copy-scale / Pool TS take a share), and the i-contraction
runs on the otherwise-idle PE: one block-delta matmul per chunk accumulates
U2 back into n-major partitions in PSUM. No adder trees on the vector
engines at all. The B_prior part of S folds into phase-1 PE matmuls
(S2 += bp.T @ U2) so the tail only handles the softmax part.
"""

import math
import numpy as np

import concourse.bacc as bacc
import concourse.bass as bass
import concourse.tile as tile
from concourse import mybir
from concourse.bass_utils import run_bass_kernel_spmd

F32 = mybir.dt.float32
F16 = mybir.dt.float16
I32 = mybir.dt.int32
AX = mybir.AxisListType
OP = mybir.AluOpType
ACTF = mybir.ActivationFunctionType

B, N, DP = 16, 1152, 8
D, DD = 10, 16
NCORES = 8
BPC = B // NCORES            # 2 batches per core
NT = N // 128                # 9 n-tiles
NG = 128 // 16               # 8 chunks of 16 n per tile
FD = D * DD                  # 160 per-batch free size (d,j)
FW = FD * DP                 # 1280 W free size per partition
FU = BPC * FD                # 320 U2 free size
NBD = BPC * D                # 20 (b,d) pairs
INV_SQRT_DP = 1.0 / math.sqrt(DP)
ROW16 = NG * FD + NBD        # W_ni 1280 | bp_bd 20

# per-(tile,chunk) product engine: 'D' (DVE TS), 'A' (ACT copy-scale),
# 'P' (Pool TS).  ACT 14 chunks, Pool 18, DVE 40.
_SEQ = ("A", "P", "D", "D", "P", "D", "D", "D")
CHUNK_ENG = {}
for _t in range(NT):
    for _g in range(NG):
        e = _SEQ[(_g + _t * 3) % NG]
        if e == "A" and _t in (3, 7):
            e = "D"
        CHUNK_ENG[(_t, _g)] = e
COPY_ENG = ("D", "A", "D", "A", "D", "A", "D", "A", "D")


def _build_kernel(tc: "tile.TileContext", out_ap, WMAIN, UAUX):
    nc = tc.nc
    with (
        tc.tile_pool(name="wpool", bufs=NT) as wpool,
        tc.tile_pool(name="ppool", bufs=3) as ppool,
        tc.tile_pool(name="tapool", bufs=4) as tapool,
        tc.tile_pool(name="persist", bufs=1) as persist,
        tc.tile_pool(name="psum_u2", bufs=3, space="PSUM") as psum_u2,
        tc.tile_pool(name="psum_s", bufs=1, space="PSUM") as psum_s,
        tc.tile_pool(name="psum_S2", bufs=1, space="PSUM") as psum_S2,
    ):
        # ---- t0 pre-work (overlaps the first DMAs) ----
        # load the exp_and_others ACT table once: covers Copy and Exp
        warm_t = persist.tile([1, 1], F32, name="warm_t")
        nc.vector.memset(warm_t[:], 0.0)
        nc.scalar.activation(warm_t[:], warm_t[:], ACTF.Exp)

        ones_t = persist.tile([128, 128], F16, name="ones_t")
        nc.vector.memset(ones_t[:], 1.0)

        # delta_g[c, p] = 1 iff p == g*16 + c//8   (c = n*8 + i)
        c_i = persist.tile([128, 1], I32, name="c_i")
        nc.gpsimd.iota(c_i[:], pattern=[[0, 1]], base=0, channel_multiplier=1)
        c8 = persist.tile([128, 1], I32, name="c8")
        nc.vector.tensor_scalar(c8[:], c_i[:], 3, None, OP.logical_shift_right)
        pcol = persist.tile([128, 128], I32, name="pcol")
        nc.gpsimd.iota(pcol[:], pattern=[[1, 128]], base=0, channel_multiplier=0)
        diff = persist.tile([128, 128], I32, name="diff")
        nc.vector.tensor_tensor(
            diff[:], pcol[:], c8[:].broadcast_to([128, 128]), OP.subtract
        )
        deltas = persist.tile([128, NG * 128], F16, name="deltas")
        for g in range(NG):
            nc.vector.tensor_scalar(
                deltas[:, g * 128:(g + 1) * 128], diff[:], g * 16, None,
                OP.is_equal,
            )

        # diag mask for phase 3 (iota trick)
        iota_t = persist.tile([NBD, FU], I32, name="iota_t")
        nc.gpsimd.iota(
            iota_t[:], pattern=[[1, NBD], [0, DD]], base=0,
            channel_multiplier=-1,
        )
        mask_t = persist.tile([NBD, FU], F32, name="mask_t")
        nc.vector.tensor_scalar(mask_t[:], iota_t[:], 0, None, OP.is_equal)

        u2_all = persist.tile([128, NT * FU], F16, name="u2_all")
        s_ps = psum_s.tile([128, FU], F32, name="s_ps")
        S2_ps = psum_S2.tile([NBD, FU], F32, name="S2_ps")

        # ---- DMAs ----
        w16 = []
        u32 = []
        for nt in range(NT):
            w_t = wpool.tile([128, ROW16], F16, tag="w16")
            u_t = wpool.tile([128, NG * BPC], F32, tag="u32")
            w16.append(w_t)
            u32.append(u_t)
            nc.sync.dma_start(w_t[:], WMAIN[nt])
            nc.sync.dma_start(u_t[:], UAUX[nt])

        with nc.allow_low_precision(reason="fp16 kernel, tol 2e-2"):
            # ---- phase 1: products (TS) + PE delta-matmul i-reduction ----
            pe_q = []      # deferred PE emission: ("d", nt) / ("s", nt) / ("b", nt)
            u2_ps_t = {}
            pp_t = {}
            for nt in range(NT):
                w_t = w16[nt]
                u_t = u32[nt]
                pp = ppool.tile([128, NG * FU], F16, tag="pp")
                pp_t[nt] = pp
                for g in range(NG):
                    eng = CHUNK_ENG[(nt, g)]
                    for b in range(BPC):
                        w_sl = w_t[:, g * FD:(g + 1) * FD]
                        o_sl = pp[:, g * FU + b * FD: g * FU + (b + 1) * FD]
                        sc = u_t[:, g * BPC + b: g * BPC + b + 1]
                        if eng == "A":
                            nc.scalar.activation(
                                o_sl, w_sl, ACTF.Copy, scale=sc)
                        elif eng == "P":
                            nc.gpsimd.tensor_scalar(
                                o_sl, w_sl, sc, None, OP.mult)
                        else:
                            nc.vector.tensor_scalar(
                                o_sl, w_sl, sc, None, OP.mult)
                u2_ps = psum_u2.tile([128, FU], F32, tag="u2ps")
                u2_ps_t[nt] = u2_ps
                pe_q.append(("d", nt))
                if nt >= 1:
                    pe_q.append(("s", nt - 1))
                    pe_q.append(("b", nt - 1))
            pe_q += [("s", NT - 1), ("b", NT - 1)]

            # emit PE + copies interleaved in dependency-friendly order
            s_first = True
            b_first = True
            copied = {}
            for kind, nt in pe_q:
                if kind == "d":
                    for g in range(NG):
                        nc.tensor.matmul(
                            u2_ps_t[nt][:],
                            deltas[:, g * 128:(g + 1) * 128],
                            pp_t[nt][:, g * FU:(g + 1) * FU],
                            start=(g == 0), stop=(g == NG - 1),
                        )
                    # copy PSUM -> SBUF fp16 (DVE/ACT alternate)
                    u2_sl = u2_all[:, nt * FU:(nt + 1) * FU]
                    if COPY_ENG[nt] == "A":
                        nc.scalar.copy(u2_sl, u2_ps_t[nt][:])
                    else:
                        nc.vector.tensor_copy(u2_sl, u2_ps_t[nt][:])
                    copied[nt] = True
                elif kind == "s":
                    nc.tensor.matmul(
                        s_ps[:], ones_t[:],
                        u2_all[:, nt * FU:(nt + 1) * FU],
                        start=s_first, stop=(nt == NT - 1),
                    )
                    s_first = False
                else:  # S2 += bp.T @ U2  (B_prior part of S, done in phase 1)
                    nc.tensor.matmul(
                        S2_ps[:],
                        w16[nt][:, NG * FD: NG * FD + NBD],
                        u2_all[:, nt * FU:(nt + 1) * FU],
                        start=b_first, stop=False,
                    )
                    b_first = False

            # ---- phase 2: 3 chunks of 3 tiles ----
            s_sb = persist.tile([128, FU], F16, name="s_sb")
            nc.vector.tensor_copy(s_sb[:], s_ps[:])

            BOUNDS = (0, 3, 6, 9)   # 3 chunks of 3 tiles
            e_all = persist.tile([128, NT * NBD], F16, name="e_all")
            z_all = persist.tile([128, NT * BPC], F16, name="z_all")
            zr_all = persist.tile([128, NT * BPC], F16, name="zr_all")
            cb_all = persist.tile([128, NT * NBD], F16, name="cb_all")

            for c in range(3):
                lo, hi = BOUNDS[c], BOUNDS[c + 1]
                NC = hi - lo
                ta = tapool.tile([128, NC * FU], F16, tag="ta")
                ta_v = ta[:].rearrange("p (t f) -> p t f", t=NC, f=FU)
                s_bc = s_sb[:].unsqueeze(1).broadcast_to([128, NC, FU])
                nc.vector.tensor_tensor(
                    ta_v, u2_all[:, lo * FU: hi * FU].rearrange(
                        "p (t f) -> p t f", t=NC, f=FU),
                    s_bc, OP.mult,
                )
                # j-reduction as a 2x-mode contiguous-halves tree; level 1 of
                # the early chunks runs on the (idle) Pool engine
                G = NC * NBD
                tg = ta[:].rearrange("p (g j) -> p g j", g=G, j=DD)
                l1 = tapool.tile([128, G * 8], F16, tag="l1")
                l1v = l1[:].rearrange("p (g j) -> p g j", g=G, j=8)
                e1 = nc.vector
                e1.tensor_tensor(l1v, tg[:, :, 0:8], tg[:, :, 8:16], OP.add)
                l2 = tapool.tile([128, G * 4], F16, tag="l2")
                l2v = l2[:].rearrange("p (g j) -> p g j", g=G, j=4)
                nc.vector.tensor_tensor(l2v, l1v[:, :, 0:4], l1v[:, :, 4:8], OP.add)
                l3 = tapool.tile([128, G * 2], F16, tag="l3")
                l3v = l3[:].rearrange("p (g j) -> p g j", g=G, j=2)
                nc.vector.tensor_tensor(l3v, l2v[:, :, 0:2], l2v[:, :, 2:4], OP.add)
                a_sl = e_all[:, lo * NBD: hi * NBD]
                nc.vector.tensor_tensor(
                    a_sl.rearrange("p (g j) -> p g j", g=G, j=1),
                    l3v[:, :, 0:1], l3v[:, :, 1:2], OP.add,
                )
                # E = exp(A / sqrt(dp))
                nc.scalar.activation(a_sl, a_sl, ACTF.Exp, scale=INV_SQRT_DP)
                # z = sum_d E ; zr = 1/z ; cb = E * zr
                z_sl = z_all[:, lo * BPC: hi * BPC]
                zr_sl = zr_all[:, lo * BPC: hi * BPC]
                nc.vector.tensor_reduce(
                    z_sl,
                    a_sl.rearrange("p (g d) -> p g d", g=NC * BPC, d=D),
                    AX.X, OP.add,
                )
                nc.vector.reciprocal(zr_sl, z_sl)
                cb_sl = cb_all[:, lo * NBD: hi * NBD]
                zr_bc = (
                    zr_sl.rearrange("p (g o) -> p g o", g=NC * BPC, o=1)
                    .broadcast_to([128, NC * BPC, D])
                )
                nc.vector.tensor_tensor(
                    cb_sl.rearrange("p (g d) -> p g d", g=NC * BPC, d=D),
                    a_sl.rearrange("p (g d) -> p g d", g=NC * BPC, d=D),
                    zr_bc, OP.mult,
                )
                for t in range(lo, hi):
                    nc.tensor.matmul(
                        S2_ps[:],
                        cb_all[:, t * NBD:(t + 1) * NBD],
                        u2_all[:, t * FU:(t + 1) * FU],
                        start=False, stop=(t == NT - 1),
                    )

            # ---- phase 3: extract diagonal (b,d)=(b',d') ----
            sm_t = persist.tile([NBD, FU], F32, name="sm_t")
            nc.vector.tensor_tensor(sm_t[:], S2_ps[:], mask_t[:], OP.mult)
            s_diag = persist.tile([NBD, DD], F32, name="s_diag")
            nc.vector.tensor_reduce(
                s_diag[:],
                sm_t[:].rearrange("p (g j) -> p j g", g=NBD, j=DD),
                AX.X, OP.add,
            )

        # ---- phase 4: squash (fp32) ----
        ss_t = persist.tile([NBD, DD], F32, name="ss_t")
        nrm2 = persist.tile([NBD, 1], F32, name="nrm2")
        nc.vector.tensor_tensor(ss_t[:], s_diag[:], s_diag[:], OP.mult)
        nc.vector.tensor_reduce(nrm2[:], ss_t[:], AX.X, OP.add)
        nrm = persist.tile([NBD, 1], F32, name="nrm")
        seed_i = persist.tile([NBD, 1], I32, name="seed_i")
        nc.vector.tensor_scalar(
            seed_i[:], nrm2[:].bitcast(I32), 1, None, OP.logical_shift_right
        )
        nc.vector.tensor_scalar(seed_i[:], seed_i[:], 0x1FBD1DF5, None, OP.add)
        seed_f = seed_i[:].bitcast(F32)
        y2 = persist.tile([NBD, 1], F32, name="y2")
        nc.vector.tensor_tensor(y2[:], seed_f, seed_f, OP.mult)
        hnum = persist.tile([NBD, 1], F32, name="hnum")
        nc.vector.scalar_tensor_tensor(hnum[:], nrm2[:], 3.0, y2[:], OP.mult, OP.add)
        hden = persist.tile([NBD, 1], F32, name="hden")
        nc.vector.scalar_tensor_tensor(hden[:], y2[:], 3.0, nrm2[:], OP.mult, OP.add)
        nwr = persist.tile([NBD, 1], F32, name="nwr")
        nc.vector.reciprocal(nwr[:], hden[:])
        nwt = persist.tile([NBD, 1], F32, name="nwt")
        nc.vector.tensor_tensor(nwt[:], hnum[:], nwr[:], OP.mult)
        nc.vector.tensor_tensor(nrm[:], seed_f, nwt[:], OP.mult)
        en = persist.tile([NBD, 1], F32, name="en")
        nc.scalar.activation(en[:], nrm[:], ACTF.Exp, scale=-1.0)
        coef = persist.tile([NBD, 1], F32, name="coef")
        nc.vector.tensor_scalar(coef[:], en[:], -1.0, 1.0, OP.mult, OP.add)
        r2 = persist.tile([NBD, 1], F32, name="r2")
        nc.vector.reciprocal(r2[:], nrm[:])
        fac = persist.tile([NBD, 1], F32, name="fac")
        nc.vector.tensor_tensor(fac[:], coef[:], r2[:], OP.mult)

        res_t = persist.tile([NBD, DD], F32, name="res_t")
        nc.vector.tensor_scalar(res_t[:], s_diag[:], fac[:], None, OP.mult)

        nc.sync.dma_start(out_ap.rearrange("b d j -> (b d) j"), res_t[:])


_CACHE: dict = {}


def _get_nc():
    if "nc" not in _CACHE:
        nc = bacc.Bacc(
            "TRN2", target_bir_lowering=False, debug=False, num_devices=NCORES
        )
        WMAIN = nc.dram_tensor(
            "wmain", [NT, 128, ROW16], F16, kind="ExternalInput"
        ).ap()
        UAUX = nc.dram_tensor(
            "uaux", [NT, 128, NG * BPC], F32, kind="ExternalInput"
        ).ap()
        out = nc.dram_tensor("out", [BPC, D, DD], F32, kind="ExternalOutput").ap()
        with tile.TileContext(nc) as tc:
            _build_kernel(tc, out, WMAIN, UAUX)
        nc.compile()
        _CACHE["nc"] = nc
    return _CACHE["nc"]


def _arrange(primary_caps, W, B_prior, core):
    """Host-side pre-arrangement into the exact SBUF tile layouts so every
    device DMA reads fully contiguous memory."""
    W = np.asarray(W, dtype=np.float32)
    Bp = np.asarray(B_prior, dtype=np.float32)
    pc = np.asarray(primary_caps, dtype=np.float32)[core * BPC:(core + 1) * BPC]
    # W[d,n,j,i] with n = nt*128 + g*16 + nn -> [nt, (nn,i), (g,d,j)]
    w_ni = (
        W.transpose(1, 3, 0, 2)               # [N, i, d, j]
        .reshape(NT, NG, 16, DP, D, DD)       # [nt, g, nn, i, d, j]
        .transpose(0, 2, 3, 1, 4, 5)          # [nt, nn, i, g, d, j]
        .reshape(NT, 128, NG * FD)
        .astype(np.float16)
    )
    # bp_bd[nt][p=n, (b,d)] = Bp[d, nt*128+p]
    bp = Bp[:, 0, :].T.reshape(NT, 128, D).astype(np.float16)   # [nt, n, d]
    bp_bd = np.broadcast_to(bp[:, :, None, :], (NT, 128, BPC, D)).reshape(
        NT, 128, NBD)
    # u32[nt][p=(nn,i), (g,b)] = u[b, nt*128+g*16+nn, i]
    u_ni = (
        pc.reshape(BPC, NT, NG, 16, DP)       # [b, nt, g, nn, i]
        .transpose(1, 3, 4, 2, 0)             # [nt, nn, i, g, b]
        .reshape(NT, 128, NG * BPC)
    )
    return {
        "wmain": np.ascontiguousarray(
            np.concatenate([w_ni, bp_bd], axis=2)),
        "uaux": np.ascontiguousarray(u_ni.astype(np.float32)),
    }


def _run(primary_caps, W, B_prior, trace=False, **kw):
    nc = _get_nc()
    in_maps = [
        _arrange(primary_caps, W, B_prior, c) for c in range(NCORES)
    ]
    res = run_bass_kernel_spmd(nc, in_maps, list(range(NCORES)), trace=trace, **kw)
    out = np.concatenate([res.results[c]["out"] for c in range(NCORES)], axis=0)
    return out.astype(np.float32), res


def kernel(primary_caps, W, B_prior):
    out, _ = _run(primary_caps, W, B_prior, trace=False)
    return out
